# revision 1
# baseline (speedup 1.0000x reference)
"""4-bit column-block-quantized linear (ColBlockQuantizedLinear) on 8 TRN2 cores.

Math:  out[b,o] = scales[o] * (sum_i inp[b,i]*wq[o,i] - zeros[o]*rowsum[b])
where wq comes from packed bytes q[o,j] (j = i//2): even i -> low nibble,
odd i -> high nibble.

Device-side identity (all O(O*I) work stays on-device):
    sum_j l*a + sum_j h*b = sum_j q*a + sum_j h*(b-16a)
with q = 16h + l, a[j]=inp[:,2j], b[j]=inp[:,2j+1].

The h-stream never materializes h as an integer: a 4-instruction uint16
bit-trick on DVE writes the bf16 BIT PATTERN 0x4300|(h<<3) (= value 128+8h,
linear in h) at 4x DVE mode.  The matmul pairs it with c' = (b-16a)/8 and the
constant 128*sum(c') falls out as a rank-1 correction row.  The q-stream is a
plain u8->bf16 cast (exact, 0..255) split across ACT and GPSIMD.  Activations
are hi/lo bf16-split so the bf16 matmuls give ~fp32 accuracy; zeros*rowsum and
the 128-offset are a K=6 correction matmul with hi/lo-split factors.

Host byte layout: per core the packed bytes [2048, 1376] are column-paired as
(m, 688+m) into uint16 [2048, 688], so the bit-trick's two output streams land
contiguously in natural column order; the q-cast output is column-interleaved
and its matmuls read it through stride-2 APs.

Sharding: column-parallel over out_features (1376 rows/core), inputs
replicated; per-core output [16,1376] gathered on host.
"""

import numpy as np
import ml_dtypes

B = 16
I = 4096
O = 11008
NCORES = 8
OS = O // NCORES          # 1376 out-features per core
HOS = OS // 2             # 688, u16-packed column count
HALF = I // 2             # 2048 packed columns
KT = HALF // 128          # 16 contraction tiles
# psum-bank o-blocks, each a single arithmetic progression in the interleaved
# q-cast layout (no block crosses the 688-column half boundary)
BLKS = [(0, 512), (512, 176), (688, 512), (1200, 176)]
N_ACT_CAST = 10           # q-cast tiles on ACT; rest on GPSIMD

BF16 = ml_dtypes.bfloat16

_CACHE = {}


def _split_hi_lo(x64):
    """Split float64 array into (hi, lo) bf16 parts: hi+lo ~= x to ~2^-17."""
    hi = x64.astype(BF16)
    lo = (x64 - hi.astype(np.float64)).astype(BF16)
    return hi, lo


def _qcast_ap(qb, s, n):
    """Stride-2 AP over the interleaved q-cast tile covering natural columns
    [s, s+n) (s,n within one half)."""
    if s < HOS:
        return qb[:, 2 * s : 2 * (s + n) : 2]
    return qb[:, 2 * (s - HOS) + 1 : 2 * (s - HOS + n) : 2]


def _build_program():
    import concourse.bacc as bacc
    import concourse.mybir as mybir
    import concourse.tile as tile

    dt = mybir.dt
    op = mybir.AluOpType
    nc = bacc.Bacc("TRN2", target_bir_lowering=False)

    q = nc.dram_tensor("q", [HALF, HOS], dt.uint16, kind="ExternalInput")
    statA = nc.dram_tensor("statA", [128, KT * 64], dt.bfloat16, kind="ExternalInput")
    statC = nc.dram_tensor("statC", [128, KT * 64], dt.bfloat16, kind="ExternalInput")
    corrL = nc.dram_tensor("corrL", [6, 64], dt.bfloat16, kind="ExternalInput")
    corrR = nc.dram_tensor("corrR", [6, OS], dt.bfloat16, kind="ExternalInput")
    sc = nc.dram_tensor("sc", [B, OS], dt.float32, kind="ExternalInput")
    out = nc.dram_tensor("out", [B, OS], dt.float32, kind="ExternalOutput")

    with tile.TileContext(nc) as tc:
        with (
            tc.tile_pool(name="consts", bufs=1) as cpool,
            tc.tile_pool(name="qp", bufs=3) as qpool,
            tc.tile_pool(name="tp", bufs=2) as tpool,
            tc.tile_pool(name="wp", bufs=3) as wpool,
            tc.tile_pool(name="op", bufs=2) as opool,
            tc.tile_pool(name="ps", bufs=1, space="PSUM") as pspool,
        ):
            statA_sb = cpool.tile([128, KT * 64], dt.bfloat16, name="statA_sb")
            statC_sb = cpool.tile([128, KT * 64], dt.bfloat16, name="statC_sb")
            corrL_sb = cpool.tile([6, 64], dt.bfloat16, name="corrL_sb")
            corrR_sb = cpool.tile([6, OS], dt.bfloat16, name="corrR_sb")
            sc_sb = cpool.tile([B, OS], dt.float32, name="sc_sb")
            nc.sync.dma_start(statA_sb, statA[:, :])
            nc.sync.dma_start(statC_sb, statC[:, :])
            nc.sync.dma_start(corrL_sb, corrL[:, :])
            nc.sync.dma_start(corrR_sb, corrR[:, :])
            nc.sync.dma_start(sc_sb, sc[:, :])

            psums = [
                pspool.tile([64, n], dt.float32, name=f"ps{i}")
                for i, (s, n) in enumerate(BLKS)
            ]

            for kt in range(KT):
                qt = qpool.tile([128, HOS], dt.uint16, name="qt", tag="qt")
                nc.sync.dma_start(qt, q[kt * 128 : (kt + 1) * 128, :])
                qb = wpool.tile([128, OS], dt.bfloat16, name="qb", tag="qb")
                hb = wpool.tile([128, OS], dt.bfloat16, name="hb", tag="hb")
                hbu = hb.bitcast(dt.uint16)
                t1 = tpool.tile([128, HOS], dt.uint16, name="t1", tag="t1")
                t2 = tpool.tile([128, HOS], dt.uint16, name="t2", tag="t2")
                # q-cast (exact bf16 of 0..255); interleaved column order
                if kt < N_ACT_CAST:
                    nc.scalar.activation(
                        qb, qt.bitcast(dt.uint8), mybir.ActivationFunctionType.Copy
                    )
                else:
                    nc.gpsimd.tensor_copy(qb, qt.bitcast(dt.uint8))
                # h-stream bit trick: bf16 bits 0x4300|(h<<3) = 128+8h
                nc.vector.tensor_scalar(t1, qt, 1, None, op.logical_shift_right)
                nc.vector.tensor_scalar(
                    hbu[:, 0:HOS], t1, 0x78, 0x4300, op.bitwise_and, op.bitwise_or
                )
                nc.vector.tensor_scalar(
                    t2, t1, 8, 0x78, op.logical_shift_right, op.bitwise_and
                )
                nc.vector.tensor_scalar(
                    hbu[:, HOS:OS], t2, 0x4300, None, op.bitwise_or
                )
                for i, (s, n) in enumerate(BLKS):
                    nc.tensor.matmul(
                        psums[i],
                        statA_sb[:, kt * 64 : kt * 64 + 64],
                        _qcast_ap(qb, s, n),
                        start=(kt == 0),
                        stop=False,
                    )
                    nc.tensor.matmul(
                        psums[i],
                        statC_sb[:, kt * 64 : kt * 64 + 64],
                        hb[:, s : s + n],
                        start=False,
                        stop=False,
                    )

            for i, (s, n) in enumerate(BLKS):
                # rank-1 corrections: -zeros*rowsum and -128*sum(c')
                nc.tensor.matmul(
                    psums[i],
                    corrL_sb,
                    corrR_sb[:, s : s + n],
                    start=False,
                    stop=True,
                )
                t0 = opool.tile([B, n], dt.float32, name="t0", tag=f"t0{i}")
                t = opool.tile([B, n], dt.float32, name="t", tag=f"t{i}")
                o = opool.tile([B, n], dt.float32, name="o", tag=f"o{i}")
                # lo-group psum -> sbuf on ACT (only one psum read allowed per TT)
                nc.scalar.activation(
                    t0, psums[i][32:48, :], mybir.ActivationFunctionType.Copy
                )
                nc.vector.tensor_tensor(t, psums[i][0:16, :], t0, op.add)
                nc.vector.tensor_tensor(o, t, sc_sb[:, s : s + n], op.mult)
                nc.sync.dma_start(out[:, s : s + n], o)

    nc.finalize()
    return nc


def _get_program():
    if "nc" not in _CACHE:
        _CACHE["nc"] = _build_program()
    return _CACHE["nc"]


def _host_prep(inp, quant_weight, scales, zeros):
    """Build per-core input maps (layout/precision prep only, no dequant math)."""
    inp64 = np.asarray(inp, dtype=np.float64)
    a = inp64[:, 0::2].T.copy()  # [HALF, B] even-i activations (pair with l)
    b = inp64[:, 1::2].T.copy()  # [HALF, B] odd-i activations (pair with h)
    # q-stream pairs with a; bit-trick h-stream pairs with c' = (b-16a)/8
    cp = (b - 16.0 * a) / 8.0
    a_hi, a_lo = _split_hi_lo(a)
    c_hi, c_lo = _split_hi_lo(cp)

    statA = np.zeros((128, KT * 64), dtype=BF16)
    statC = np.zeros((128, KT * 64), dtype=BF16)
    for kt in range(KT):
        rows = slice(kt * 128, (kt + 1) * 128)
        statA[:, kt * 64 : kt * 64 + 16] = a_hi[rows]
        statA[:, kt * 64 + 32 : kt * 64 + 48] = a_lo[rows]
        statC[:, kt * 64 : kt * 64 + 16] = c_hi[rows]
        statC[:, kt * 64 + 32 : kt * 64 + 48] = c_lo[rows]

    rowsum = inp64.sum(axis=1)  # [B]
    rs_hi, rs_lo = _split_hi_lo(rowsum)
    s_c = cp.sum(axis=0)  # [B]  sum_j c'[j,b]
    sc_hi, sc_lo = _split_hi_lo(s_c)
    corrL = np.zeros((6, 64), dtype=BF16)
    corrL[0, :16] = rs_hi
    corrL[1, :16] = rs_hi
    corrL[2, :16] = rs_lo
    corrL[3, :16] = rs_lo
    corrL[4, :16] = sc_hi
    corrL[5, :16] = sc_lo

    qw = np.asarray(quant_weight)
    scales = np.asarray(scales, dtype=np.float64).reshape(-1)
    zeros = np.asarray(zeros, dtype=np.float64).reshape(-1)

    in_maps = []
    for cidx in range(NCORES):
        rows = slice(cidx * OS, (cidx + 1) * OS)
        qc = qw[rows].astype(np.uint8).T  # [HALF, OS] natural columns
        # byte-pair columns (m, 688+m) -> uint16 elements
        qc2 = np.empty((HALF, OS), dtype=np.uint8)
        qc2[:, 0::2] = qc[:, :HOS]
        qc2[:, 1::2] = qc[:, HOS:]
        qu16 = np.ascontiguousarray(qc2).view(np.uint16)  # [HALF, HOS]
        z = zeros[rows]
        z_hi, z_lo = _split_hi_lo(z)
        corrR = np.zeros((6, OS), dtype=BF16)
        corrR[0] = -z_hi
        corrR[1] = -z_lo
        corrR[2] = -z_hi
        corrR[3] = -z_lo
        corrR[4] = -128.0
        corrR[5] = -128.0
        sc_c = np.broadcast_to(scales[rows].astype(np.float32), (B, OS)).copy()
        in_maps.append(
            {
                "q": qu16,
                "statA": statA,
                "statC": statC,
                "corrL": corrL,
                "corrR": corrR,
                "sc": sc_c,
            }
        )
    return in_maps


def kernel(inp, quant_weight, scales, zeros):
    from concourse.bass_utils import run_bass_kernel_spmd

    nc = _get_program()
    in_maps = _host_prep(inp, quant_weight, scales, zeros)
    res = run_bass_kernel_spmd(nc, in_maps, core_ids=list(range(NCORES)))
    out = np.concatenate(
        [res.results[c]["out"] for c in range(NCORES)], axis=1
    )
    return np.ascontiguousarray(out.astype(np.float32))



# revision 6
# speedup vs baseline: 2.5703x; 2.5703x over previous
"""4-bit column-block-quantized linear on 8 TRN2 cores — fp8 DoubleRow version.

Math:  out[b,o] = scales[o] * (sum_i inp[b,i]*wq[o,i] - zeros[o]*rowsum[b])
where wq nibbles come from packed bytes q[o,j] (j = i//2): even i -> low
nibble, odd i -> high nibble.

Device-side scheme (all O(O*I) work on-device):
  * The packed bytes stream through the PE as float8e4 (e4m3, bias 7).
    Nibble bit patterns 0x0..0xF ARE e4m3 values nibble*2^-9 (subnormals are
    linear), so unpacking is just 2 DVE tensor_scalar ops per 256-row tile:
        l = q & 0x0F0F0F0F          (low nibbles,  pairs even-i activations)
        h = (q >> 4) & 0x0F0F0F0F   (high nibbles, pairs odd-i activations)
    done on uint32 views (single-src ops -> DVE 2x_2p mode).  The 2^9 factor
    is folded into the final scales multiply; with USE_OFFSET the nibbles are
    biased +8 into normal range instead and the bias folds into the rank-1
    correction.
  * Matmuls run fp8 with perf_mode=DoubleRow: one matmul contracts 256 rows
    (two 128-row k-tiles), stationary = activations split hi/lo in e4m3
    (psum rows 0:16 hi, 16:32 lo), moving = the nibble streams.
  * -zeros*rowsum lands via a K=4 bf16 rank-1 correction matmul issued first
    (keeps the PE busy during the initial DMA).
  * Tail per psum block: ACT copies lo rows to SBUF, DVE adds hi rows, DVE
    multiplies by 512*scales, DMA out.

Sharding: column-parallel over out_features (1376 rows/core), inputs
replicated; per-core output [16,1376] gathered on host.
"""

import numpy as np
import ml_dtypes

B = 16
I = 4096
O = 11008
NCORES = 8
OS = O // NCORES          # 1376 out-features per core
HALF = I // 2             # 2048 packed columns (j)
NDKT = 8                  # double-k-tiles of 256 j-rows each
BLKS = [(0, 512), (512, 512), (1024, 352)]  # psum-bank o-blocks

USE_OFFSET = False        # True: bias nibbles +8 (normal-range e4m3) instead
                          # of relying on PE subnormal handling

BF16 = ml_dtypes.bfloat16
FP8 = ml_dtypes.float8_e4m3fn

_CACHE = {}


def _split_bf16(x64):
    hi = x64.astype(BF16)
    lo = (x64 - hi.astype(np.float64)).astype(BF16)
    return hi, lo


def _split_fp8(x64):
    hi = x64.astype(FP8)
    lo = (x64 - hi.astype(np.float64)).astype(FP8)
    return hi, lo


def _build_program():
    import concourse.bacc as bacc
    import concourse.mybir as mybir
    import concourse.tile as tile

    dt = mybir.dt
    op = mybir.AluOpType
    pm = mybir.MatmulPerfMode
    nc = bacc.Bacc("TRN2", target_bir_lowering=False)

    q = nc.dram_tensor("q", [NDKT * 128, 688], dt.uint32, kind="ExternalInput")
    statA = nc.dram_tensor("statA", [128, NDKT * 128], dt.float8e4, kind="ExternalInput")
    statB = nc.dram_tensor("statB", [128, NDKT * 128], dt.float8e4, kind="ExternalInput")
    corrL = nc.dram_tensor("corrL", [4, 64], dt.bfloat16, kind="ExternalInput")
    corrR = nc.dram_tensor("corrR", [4, OS], dt.bfloat16, kind="ExternalInput")
    sc = nc.dram_tensor("sc", [B, OS], dt.float32, kind="ExternalInput")
    out = nc.dram_tensor("out", [B, OS], dt.float32, kind="ExternalOutput")

    with tile.TileContext(nc) as tc:
        with (
            tc.tile_pool(name="consts", bufs=1) as cpool,
            tc.tile_pool(name="qp", bufs=3) as qpool,
            tc.tile_pool(name="wp", bufs=2) as wpool,
            tc.tile_pool(name="op", bufs=2) as opool,
            tc.tile_pool(name="ps", bufs=1, space="PSUM") as pspool,
        ):
            statA_sb = cpool.tile([128, NDKT * 128], dt.float8e4, name="statA_sb")
            statB_sb = cpool.tile([128, NDKT * 128], dt.float8e4, name="statB_sb")
            corrL_sb = cpool.tile([4, 64], dt.bfloat16, name="corrL_sb")
            corrR_sb = cpool.tile([4, OS], dt.bfloat16, name="corrR_sb")
            sc_sb = cpool.tile([B, OS], dt.float32, name="sc_sb")
            nc.sync.dma_start(corrL_sb, corrL[:, :])
            nc.sync.dma_start(corrR_sb, corrR[:, :])
            nc.sync.dma_start(statA_sb, statA[:, :])
            nc.sync.dma_start(statB_sb, statB[:, :])
            nc.sync.dma_start(sc_sb, sc[:, :])

            psums = [
                pspool.tile([64, n], dt.float32, name=f"ps{i}")
                for i, (s, n) in enumerate(BLKS)
            ]

            # rank-1 correction first: PE has work while q tiles stream in
            for i, (s, n) in enumerate(BLKS):
                nc.tensor.matmul(
                    psums[i], corrL_sb, corrR_sb[:, s : s + n],
                    start=True, stop=False,
                )

            for d in range(NDKT):
                qt = qpool.tile([128, 688], dt.uint32, name="qt", tag="qt")
                nc.sync.dma_start(qt, q[d * 128 : (d + 1) * 128, :])
                lb = wpool.tile([128, 688], dt.uint32, name="lb", tag="lb")
                hb = wpool.tile([128, 688], dt.uint32, name="hb", tag="hb")
                if USE_OFFSET:
                    # 0x50|d is e4m3 for 8+d: nibbles biased into normal range
                    nc.vector.tensor_scalar(
                        lb, qt, 0x0F0F0F0F, 0x50505050, op.bitwise_and, op.bitwise_or
                    )
                    nc.vector.tensor_scalar(
                        hb, qt, 4, 0x0F0F0F0F, op.logical_shift_right, op.bitwise_and
                    )
                    nc.vector.tensor_scalar(
                        hb, hb, 0x50505050, None, op.bitwise_or
                    )
                else:
                    nc.vector.tensor_scalar(
                        lb, qt, 0x0F0F0F0F, None, op.bitwise_and
                    )
                    nc.vector.tensor_scalar(
                        hb, qt, 4, 0x0F0F0F0F, op.logical_shift_right, op.bitwise_and
                    )
                last = d == NDKT - 1
                lb8 = lb.bitcast(dt.float8e4).rearrange("p (g n) -> p g n", g=2)
                hb8 = hb.bitcast(dt.float8e4).rearrange("p (g n) -> p g n", g=2)
                sA = statA_sb[:, d * 128 : (d + 1) * 128].rearrange(
                    "p (g m) -> p g m", g=2
                )
                sB = statB_sb[:, d * 128 : (d + 1) * 128].rearrange(
                    "p (g m) -> p g m", g=2
                )
                for i, (s, n) in enumerate(BLKS):
                    nc.tensor.matmul(
                        psums[i], sA, lb8[:, :, s : s + n],
                        start=False, stop=False, perf_mode=pm.DoubleRow,
                    )
                for i, (s, n) in enumerate(BLKS):
                    nc.tensor.matmul(
                        psums[i], sB, hb8[:, :, s : s + n],
                        start=False, stop=last, perf_mode=pm.DoubleRow,
                    )

            for i, (s, n) in enumerate(BLKS):
                t0 = opool.tile([B, n], dt.float32, name="t0", tag=f"t0{i}")
                t1 = opool.tile([B, n], dt.float32, name="t1", tag=f"t1{i}")
                o = opool.tile([B, n], dt.float32, name="o", tag=f"o{i}")
                # lo-group psum -> sbuf on ACT (one psum read per DVE TT max)
                nc.scalar.activation(
                    t0, psums[i][32:48, :], mybir.ActivationFunctionType.Copy
                )
                nc.vector.tensor_tensor(t1, psums[i][0:16, :], t0, op.add)
                nc.vector.tensor_tensor(o, t1, sc_sb[:, s : s + n], op.mult)
                nc.sync.dma_start(out[:, s : s + n], o)

    nc.finalize()
    return nc


def _get_program():
    if "nc" not in _CACHE:
        _CACHE["nc"] = _build_program()
    return _CACHE["nc"]


def _host_prep(inp, quant_weight, scales, zeros):
    """Build per-core input maps (layout/precision prep, no dequant math)."""
    inp64 = np.asarray(inp, dtype=np.float64)
    a = np.ascontiguousarray(inp64[:, 0::2].T)  # [HALF, B] even-i (pairs l)
    b = np.ascontiguousarray(inp64[:, 1::2].T)  # [HALF, B] odd-i  (pairs h)
    a_hi, a_lo = _split_fp8(a)
    b_hi, b_lo = _split_fp8(b)

    def stat(hi, lo):
        # [HALF,B] -> [128, NDKT*2*64]: per dkt d, group g, cols
        # [hi(16) 0(16) lo(16) 0(16)] of j-rows d*256 + g*128 + p
        # (psum partition slices must be 32-aligned -> hi rows 0:16, lo 32:48)
        z = np.zeros((NDKT, 2, 128, B), dtype=FP8)
        m = np.concatenate(
            [hi.reshape(NDKT, 2, 128, B), z, lo.reshape(NDKT, 2, 128, B), z],
            axis=-1,
        )  # [NDKT, 2, 128, 64]
        return np.ascontiguousarray(
            m.transpose(2, 0, 1, 3).reshape(128, NDKT * 128)
        )

    statA = stat(a_hi, a_lo)
    statB = stat(b_hi, b_lo)

    rowsum = inp64.sum(axis=1)  # [B]
    rs_hi, rs_lo = _split_bf16(rowsum)
    corrL = np.zeros((4, 64), dtype=BF16)
    # stream values are nibble*2^-9 (subnormal path) or nibble+8 (offset
    # path); psum is scaled by P = 2^-9 or 1 accordingly
    s9 = np.float64(1.0 if USE_OFFSET else 2.0**-9)
    corrL[0, :B] = (rs_hi.astype(np.float64) * s9).astype(BF16)
    corrL[1, :B] = corrL[0, :B]
    corrL[2, :B] = (rs_lo.astype(np.float64) * s9).astype(BF16)
    corrL[3, :B] = corrL[2, :B]

    qw = np.asarray(quant_weight)
    scales = np.asarray(scales, dtype=np.float64).reshape(-1)
    zeros = np.asarray(zeros, dtype=np.float64).reshape(-1)

    in_maps = []
    for cidx in range(NCORES):
        rows = slice(cidx * OS, (cidx + 1) * OS)
        qc = np.ascontiguousarray(qw[rows].astype(np.uint8).T)  # [HALF, OS]
        q_arr = np.ascontiguousarray(
            qc.reshape(NDKT, 2, 128, OS).transpose(0, 2, 1, 3).reshape(
                NDKT * 128, 2 * OS
            )
        ).view(np.uint32)
        z8 = zeros[rows] + (8.0 if USE_OFFSET else 0.0)
        z_hi, z_lo = _split_bf16(z8)
        corrR = np.zeros((4, OS), dtype=BF16)
        corrR[0] = -z_hi
        corrR[1] = -z_lo
        corrR[2] = -z_hi
        corrR[3] = -z_lo
        sc_c = np.broadcast_to(
            (scales[rows] * (1.0 if USE_OFFSET else 512.0)).astype(np.float32),
            (B, OS),
        ).copy()
        in_maps.append(
            {
                "q": q_arr,
                "statA": statA,
                "statB": statB,
                "corrL": corrL,
                "corrR": corrR,
                "sc": sc_c,
            }
        )
    return in_maps


def kernel(inp, quant_weight, scales, zeros):
    from concourse.bass_utils import run_bass_kernel_spmd

    nc = _get_program()
    in_maps = _host_prep(inp, quant_weight, scales, zeros)
    res = run_bass_kernel_spmd(nc, in_maps, core_ids=list(range(NCORES)))
    out = np.concatenate(
        [res.results[c]["out"] for c in range(NCORES)], axis=1
    )
    return np.ascontiguousarray(out.astype(np.float32))


# revision 7
# speedup vs baseline: 2.6190x; 1.0189x over previous
"""4-bit column-block-quantized linear on 8 TRN2 cores — fp8 DoubleRow version.

Math:  out[b,o] = scales[o] * (sum_i inp[b,i]*wq[o,i] - zeros[o]*rowsum[b])
where wq nibbles come from packed bytes q[o,j] (j = i//2): even i -> low
nibble, odd i -> high nibble.

Device-side scheme (all O(O*I) work on-device):
  * The packed bytes stream through the PE as float8e4 (e4m3, bias 7).
    Nibble bit patterns 0x0..0xF ARE e4m3 values nibble*2^-9 (subnormals are
    linear), so unpacking is just 2 DVE tensor_scalar ops per 256-row tile:
        l = q & 0x0F0F0F0F          (low nibbles,  pairs even-i activations)
        h = (q >> 4) & 0x0F0F0F0F   (high nibbles, pairs odd-i activations)
    done on uint32 views (single-src ops -> DVE 2x_2p mode).  The 2^9 factor
    is folded into the final scales multiply; with USE_OFFSET the nibbles are
    biased +8 into normal range instead and the bias folds into the rank-1
    correction.
  * Matmuls run fp8 with perf_mode=DoubleRow: one matmul contracts 256 rows
    (two 128-row k-tiles), stationary = activations split hi/lo in e4m3
    (psum rows 0:16 hi, 16:32 lo), moving = the nibble streams.
  * -zeros*rowsum lands via a K=4 bf16 rank-1 correction matmul issued first
    (keeps the PE busy during the initial DMA).
  * Tail per psum block: ACT copies lo rows to SBUF, DVE adds hi rows, DVE
    multiplies by 512*scales, DMA out.

Sharding: column-parallel over out_features (1376 rows/core), inputs
replicated; per-core output [16,1376] gathered on host.
"""

import numpy as np
import ml_dtypes

B = 16
I = 4096
O = 11008
NCORES = 8
OS = O // NCORES          # 1376 out-features per core
HALF = I // 2             # 2048 packed columns (j)
NDKT = 8                  # double-k-tiles of 256 j-rows each
BLKS = [(0, 512), (512, 512), (1024, 352)]  # psum-bank o-blocks

USE_OFFSET = False        # True: bias nibbles +8 (normal-range e4m3) instead
                          # of relying on PE subnormal handling

BF16 = ml_dtypes.bfloat16
FP8 = ml_dtypes.float8_e4m3fn

_CACHE = {}


def _split_bf16(x64):
    hi = x64.astype(BF16)
    lo = (x64 - hi.astype(np.float64)).astype(BF16)
    return hi, lo


def _split_fp8(x64):
    hi = x64.astype(FP8)
    lo = (x64 - hi.astype(np.float64)).astype(FP8)
    return hi, lo


def _build_program():
    import concourse.bacc as bacc
    import concourse.mybir as mybir
    import concourse.tile as tile

    dt = mybir.dt
    op = mybir.AluOpType
    pm = mybir.MatmulPerfMode
    nc = bacc.Bacc("TRN2", target_bir_lowering=False)

    q = nc.dram_tensor("q", [NDKT * 128, 688], dt.uint32, kind="ExternalInput")
    stat = nc.dram_tensor("stat", [128, NDKT * 256], dt.float8e4, kind="ExternalInput")
    corr = nc.dram_tensor("corr", [4, 64 + OS], dt.bfloat16, kind="ExternalInput")
    sc = nc.dram_tensor("sc", [B, OS], dt.float32, kind="ExternalInput")
    out = nc.dram_tensor("out", [B, OS], dt.float32, kind="ExternalOutput")

    with tile.TileContext(nc) as tc:
        with (
            tc.tile_pool(name="consts", bufs=1) as cpool,
            tc.tile_pool(name="qp", bufs=3) as qpool,
            tc.tile_pool(name="wp", bufs=2) as wpool,
            tc.tile_pool(name="op", bufs=2) as opool,
            tc.tile_pool(name="ps", bufs=1, space="PSUM") as pspool,
        ):
            stat_sb = cpool.tile([128, NDKT * 256], dt.float8e4, name="stat_sb")
            corr_sb = cpool.tile([4, 64 + OS], dt.bfloat16, name="corr_sb")
            sc_sb = cpool.tile([B, OS], dt.float32, name="sc_sb")
            nc.sync.dma_start(corr_sb, corr[:, :])
            nc.sync.dma_start(stat_sb, stat[:, :])
            corrL_sb = corr_sb[:, 0:64]
            corrR_sb = corr_sb[:, 64 : 64 + OS]

            psums = [
                pspool.tile([64, n], dt.float32, name=f"ps{i}")
                for i, (s, n) in enumerate(BLKS)
            ]

            # rank-1 correction first: PE has work while q tiles stream in
            for i, (s, n) in enumerate(BLKS):
                nc.tensor.matmul(
                    psums[i], corrL_sb, corrR_sb[:, s : s + n],
                    start=True, stop=False,
                )

            for d in range(NDKT):
                qt = qpool.tile([128, 688], dt.uint32, name="qt", tag="qt")
                nc.sync.dma_start(qt, q[d * 128 : (d + 1) * 128, :])
                lb = wpool.tile([128, 688], dt.uint32, name="lb", tag="lb")
                hb = wpool.tile([128, 688], dt.uint32, name="hb", tag="hb")
                if USE_OFFSET:
                    # 0x50|d is e4m3 for 8+d: nibbles biased into normal range
                    nc.vector.tensor_scalar(
                        lb, qt, 0x0F0F0F0F, 0x50505050, op.bitwise_and, op.bitwise_or
                    )
                    nc.vector.tensor_scalar(
                        hb, qt, 4, 0x0F0F0F0F, op.logical_shift_right, op.bitwise_and
                    )
                    nc.vector.tensor_scalar(
                        hb, hb, 0x50505050, None, op.bitwise_or
                    )
                else:
                    nc.vector.tensor_scalar(
                        lb, qt, 0x0F0F0F0F, None, op.bitwise_and
                    )
                    nc.vector.tensor_scalar(
                        hb, qt, 4, 0x0F0F0F0F, op.logical_shift_right, op.bitwise_and
                    )
                last = d == NDKT - 1
                # byte pairs (groupA[n], groupB[n]) are host-interleaved
                # adjacently so the DR pair fetch is a contiguous 2-byte read
                lb8 = lb.bitcast(dt.float8e4).rearrange("p (n g) -> p g n", g=2)
                hb8 = hb.bitcast(dt.float8e4).rearrange("p (n g) -> p g n", g=2)
                sA = stat_sb[:, d * 256 : d * 256 + 128].rearrange(
                    "p (g m) -> p g m", g=2
                )
                sB = stat_sb[:, d * 256 + 128 : d * 256 + 256].rearrange(
                    "p (g m) -> p g m", g=2
                )
                for i, (s, n) in enumerate(BLKS):
                    nc.tensor.matmul(
                        psums[i], sA, lb8[:, :, s : s + n],
                        start=False, stop=False, perf_mode=pm.DoubleRow,
                    )
                for i, (s, n) in enumerate(BLKS):
                    nc.tensor.matmul(
                        psums[i], sB, hb8[:, :, s : s + n],
                        start=False, stop=last, perf_mode=pm.DoubleRow,
                    )

            nc.sync.dma_start(sc_sb, sc[:, :])
            for i, (s, n) in enumerate(BLKS):
                t0 = opool.tile([B, n], dt.float32, name="t0", tag=f"t0{i}")
                t1 = opool.tile([B, n], dt.float32, name="t1", tag=f"t1{i}")
                o = opool.tile([B, n], dt.float32, name="o", tag=f"o{i}")
                # lo-group psum -> sbuf on ACT (one psum read per DVE TT max)
                nc.scalar.activation(
                    t0, psums[i][32:48, :], mybir.ActivationFunctionType.Copy
                )
                nc.vector.tensor_tensor(t1, psums[i][0:16, :], t0, op.add)
                nc.vector.tensor_tensor(o, t1, sc_sb[:, s : s + n], op.mult)
                nc.sync.dma_start(out[:, s : s + n], o)

    nc.finalize()
    return nc


def _get_program():
    if "nc" not in _CACHE:
        _CACHE["nc"] = _build_program()
    return _CACHE["nc"]


def _host_prep(inp, quant_weight, scales, zeros):
    """Build per-core input maps (layout/precision prep, no dequant math)."""
    inp64 = np.asarray(inp, dtype=np.float64)
    a = np.ascontiguousarray(inp64[:, 0::2].T)  # [HALF, B] even-i (pairs l)
    b = np.ascontiguousarray(inp64[:, 1::2].T)  # [HALF, B] odd-i  (pairs h)
    a_hi, a_lo = _split_fp8(a)
    b_hi, b_lo = _split_fp8(b)

    def stat(hi, lo):
        # [HALF,B] -> [128, NDKT*2*64]: per dkt d, group g, cols
        # [hi(16) 0(16) lo(16) 0(16)] of j-rows d*256 + g*128 + p
        # (psum partition slices must be 32-aligned -> hi rows 0:16, lo 32:48)
        z = np.zeros((NDKT, 2, 128, B), dtype=FP8)
        m = np.concatenate(
            [hi.reshape(NDKT, 2, 128, B), z, lo.reshape(NDKT, 2, 128, B), z],
            axis=-1,
        )  # [NDKT, 2, 128, 64]
        return np.ascontiguousarray(
            m.transpose(2, 0, 1, 3).reshape(128, NDKT * 128)
        )

    statA = stat(a_hi, a_lo)
    statB = stat(b_hi, b_lo)
    # merged [128, NDKT*256]: per dkt, statA 128 cols then statB 128 cols
    stat_m = np.ascontiguousarray(
        np.concatenate(
            [statA.reshape(128, NDKT, 128), statB.reshape(128, NDKT, 128)],
            axis=-1,
        ).reshape(128, NDKT * 256)
    )

    rowsum = inp64.sum(axis=1)  # [B]
    rs_hi, rs_lo = _split_bf16(rowsum)
    corrL = np.zeros((4, 64), dtype=BF16)
    # stream values are nibble*2^-9 (subnormal path) or nibble+8 (offset
    # path); psum is scaled by P = 2^-9 or 1 accordingly
    s9 = np.float64(1.0 if USE_OFFSET else 2.0**-9)
    corrL[0, :B] = (rs_hi.astype(np.float64) * s9).astype(BF16)
    corrL[1, :B] = corrL[0, :B]
    corrL[2, :B] = (rs_lo.astype(np.float64) * s9).astype(BF16)
    corrL[3, :B] = corrL[2, :B]

    qw = np.asarray(quant_weight)
    scales = np.asarray(scales, dtype=np.float64).reshape(-1)
    zeros = np.asarray(zeros, dtype=np.float64).reshape(-1)

    in_maps = []
    for cidx in range(NCORES):
        rows = slice(cidx * OS, (cidx + 1) * OS)
        qc = np.ascontiguousarray(qw[rows].astype(np.uint8).T)  # [HALF, OS]
        # byte layout per partition: (gA[c0], gB[c0], gA[c1], gB[c1], ...)
        q_arr = np.ascontiguousarray(
            qc.reshape(NDKT, 2, 128, OS).transpose(0, 2, 3, 1).reshape(
                NDKT * 128, 2 * OS
            )
        ).view(np.uint32)
        z8 = zeros[rows] + (8.0 if USE_OFFSET else 0.0)
        z_hi, z_lo = _split_bf16(z8)
        corr_m = np.zeros((4, 64 + OS), dtype=BF16)
        corr_m[:, :64] = corrL
        corr_m[0, 64:] = -z_hi
        corr_m[1, 64:] = -z_lo
        corr_m[2, 64:] = -z_hi
        corr_m[3, 64:] = -z_lo
        sc_c = np.broadcast_to(
            (scales[rows] * (1.0 if USE_OFFSET else 512.0)).astype(np.float32),
            (B, OS),
        ).copy()
        in_maps.append({"q": q_arr, "stat": stat_m, "corr": corr_m, "sc": sc_c})
    return in_maps


def kernel(inp, quant_weight, scales, zeros):
    from concourse.bass_utils import run_bass_kernel_spmd

    nc = _get_program()
    in_maps = _host_prep(inp, quant_weight, scales, zeros)
    res = run_bass_kernel_spmd(nc, in_maps, core_ids=list(range(NCORES)))
    out = np.concatenate(
        [res.results[c]["out"] for c in range(NCORES)], axis=1
    )
    return np.ascontiguousarray(out.astype(np.float32))


# revision 10
# speedup vs baseline: 2.6941x; 1.0287x over previous
"""4-bit column-block-quantized linear on 8 TRN2 cores — fp8 DoubleRow version.

Math:  out[b,o] = scales[o] * (sum_i inp[b,i]*wq[o,i] - zeros[o]*rowsum[b])
where wq nibbles come from packed bytes q[o,j] (j = i//2): even i -> low
nibble, odd i -> high nibble.

Device-side scheme (all O(O*I) work on-device):
  * The packed bytes stream through the PE as float8e4 (e4m3, bias 7).
    Nibble bit patterns 0x0..0xF ARE e4m3 values nibble*2^-9 (subnormals are
    linear), so unpacking is just 2 DVE tensor_scalar ops per 256-row tile:
        l = q & 0x0F0F0F0F          (low nibbles,  pairs even-i activations)
        h = (q >> 4) & 0x0F0F0F0F   (high nibbles, pairs odd-i activations)
    done on uint32 views (single-src ops -> DVE 2x_2p mode).  The 2^9 factor
    is folded into the final scales multiply; with USE_OFFSET the nibbles are
    biased +8 into normal range instead and the bias folds into the rank-1
    correction.
  * Matmuls run fp8 with perf_mode=DoubleRow: one matmul contracts 256 rows
    (two 128-row k-tiles), stationary = activations split hi/lo in e4m3
    (psum rows 0:16 hi, 16:32 lo), moving = the nibble streams.
  * -zeros*rowsum lands via a K=4 bf16 rank-1 correction matmul issued first
    (keeps the PE busy during the initial DMA).
  * Tail per psum block: ACT copies lo rows to SBUF, DVE adds hi rows, DVE
    multiplies by 512*scales, DMA out.

Sharding: column-parallel over out_features (1376 rows/core), inputs
replicated; per-core output [16,1376] gathered on host.
"""

import numpy as np
import ml_dtypes

B = 16
I = 4096
O = 11008
NCORES = 8
OS = O // NCORES          # 1376 out-features per core
HALF = I // 2             # 2048 packed columns (j)
NDKT = 8                  # double-k-tiles of 256 j-rows each
BLKS = [(0, 512), (512, 512), (1024, 352)]  # psum-bank o-blocks

USE_OFFSET = False        # True: bias nibbles +8 (normal-range e4m3) instead
                          # of relying on PE subnormal handling

BF16 = ml_dtypes.bfloat16
FP8 = ml_dtypes.float8_e4m3fn

_CACHE = {}


def _split_bf16(x64):
    hi = x64.astype(BF16)
    lo = (x64 - hi.astype(np.float64)).astype(BF16)
    return hi, lo


def _split_fp8(x64):
    hi = x64.astype(FP8)
    lo = (x64 - hi.astype(np.float64)).astype(FP8)
    return hi, lo


def _build_program():
    import concourse.bacc as bacc
    import concourse.mybir as mybir
    import concourse.tile as tile

    dt = mybir.dt
    op = mybir.AluOpType
    pm = mybir.MatmulPerfMode
    nc = bacc.Bacc("TRN2", target_bir_lowering=False)

    q = nc.dram_tensor("q", [NDKT * 128, 688], dt.uint32, kind="ExternalInput")
    stat = nc.dram_tensor("stat", [128, NDKT * 256], dt.float8e4, kind="ExternalInput")
    corr = nc.dram_tensor("corr", [4, 64 + OS], dt.bfloat16, kind="ExternalInput")
    sc = nc.dram_tensor("sc", [B, OS], dt.float32, kind="ExternalInput")
    out = nc.dram_tensor("out", [B, OS], dt.float32, kind="ExternalOutput")

    with tile.TileContext(nc) as tc:
        with (
            tc.tile_pool(name="consts", bufs=1) as cpool,
            tc.tile_pool(name="qp", bufs=3) as qpool,
            tc.tile_pool(name="wp", bufs=2) as wpool,
            tc.tile_pool(name="op", bufs=2) as opool,
            tc.tile_pool(name="ps", bufs=1, space="PSUM") as pspool,
        ):
            stat_sb = cpool.tile([128, NDKT * 256], dt.float8e4, name="stat_sb")
            corr_sb = cpool.tile([4, 64 + OS], dt.bfloat16, name="corr_sb")
            sc_sb = cpool.tile([B, OS], dt.float32, name="sc_sb")
            corrL_sb = corr_sb[:, 0:64]
            corrR_sb = corr_sb[:, 64 : 64 + OS]

            psums = [
                pspool.tile([64, n], dt.float32, name=f"ps{i}")
                for i, (s, n) in enumerate(BLKS)
            ]

            # q quad-tiles (2 double-k-tiles each): DMA first so DVE can start
            NQD = NDKT // 2
            qts = []
            qtiles = []
            for qd in range(NQD):
                qt = qpool.tile([128, 1376], dt.uint32, name="qt", tag="qt")
                qtiles.append(qt)
                src = q[qd * 256 : (qd + 1) * 256, :].rearrange(
                    "(t p) c -> p t c", t=2
                )
                qt3 = qt.rearrange("p (t c) -> p t c", t=2)
                if qd == 0:
                    nc.sync.dma_start(qt3, src)
                qts.append((qt3, src))

            nc.sync.dma_start(corr_sb, corr[:, :])
            nc.sync.dma_start(stat_sb, stat[:, :])
            # rank-1 correction first: PE has work while q tiles stream in
            for i, (s, n) in enumerate(BLKS):
                nc.tensor.matmul(
                    psums[i], corrL_sb, corrR_sb[:, s : s + n],
                    start=True, stop=False,
                )

            def tail(i, s, n):
                t0 = opool.tile([B, n], dt.float32, name="t0", tag=f"t0{i}")
                t1 = opool.tile([B, n], dt.float32, name="t1", tag=f"t1{i}")
                o = opool.tile([B, n], dt.float32, name="o", tag=f"o{i}")
                # lo-group psum -> sbuf on ACT (one psum read per DVE TT max)
                nc.scalar.activation(
                    t0, psums[i][32:48, :], mybir.ActivationFunctionType.Copy
                )
                nc.vector.tensor_tensor(t1, psums[i][0:16, :], t0, op.add)
                nc.vector.tensor_tensor(o, t1, sc_sb[:, s : s + n], op.mult)
                nc.sync.dma_start(out[:, s : s + n], o)

            for qd in range(NQD):
                qt3, src = qts[qd]
                qt = qtiles[qd]
                if qd > 0:
                    nc.sync.dma_start(qt3, src)
                if qd == 0:
                    nc.sync.dma_start(sc_sb, sc[:, :])
                lb = wpool.tile([128, 1376], dt.uint32, name="lb", tag="lb")
                hb = wpool.tile([128, 1376], dt.uint32, name="hb", tag="hb")
                if USE_OFFSET:
                    # 0x50|d is e4m3 for 8+d: nibbles biased into normal range
                    nc.vector.tensor_scalar(
                        lb, qt, 0x0F0F0F0F, 0x50505050, op.bitwise_and, op.bitwise_or
                    )
                    nc.vector.tensor_scalar(
                        hb, qt, 4, 0x0F0F0F0F, op.logical_shift_right, op.bitwise_and
                    )
                    nc.vector.tensor_scalar(
                        hb, hb, 0x50505050, None, op.bitwise_or
                    )
                else:
                    nc.vector.tensor_scalar(
                        lb, qt, 0x0F0F0F0F, None, op.bitwise_and
                    )
                    nc.vector.tensor_scalar(
                        hb, qt, 4, 0x0F0F0F0F, op.logical_shift_right, op.bitwise_and
                    )
                lb8 = lb.bitcast(dt.float8e4)
                hb8 = hb.bitcast(dt.float8e4)
                for j in range(2):
                    d = 2 * qd + j
                    # byte pairs (groupA[n], groupB[n]) host-interleaved
                    lbd = lb8[:, j * 2752 : (j + 1) * 2752].rearrange(
                        "p (n g) -> p g n", g=2
                    )
                    hbd = hb8[:, j * 2752 : (j + 1) * 2752].rearrange(
                        "p (n g) -> p g n", g=2
                    )
                    sA = stat_sb[:, d * 256 : d * 256 + 128].rearrange(
                        "p (g m) -> p g m", g=2
                    )
                    sB = stat_sb[:, d * 256 + 128 : d * 256 + 256].rearrange(
                        "p (g m) -> p g m", g=2
                    )
                    if d < NDKT - 1:
                        for i, (s, n) in enumerate(BLKS):
                            nc.tensor.matmul(
                                psums[i], sA, lbd[:, :, s : s + n],
                                start=False, stop=False, perf_mode=pm.DoubleRow,
                            )
                        for i, (s, n) in enumerate(BLKS):
                            nc.tensor.matmul(
                                psums[i], sB, hbd[:, :, s : s + n],
                                start=False, stop=False, perf_mode=pm.DoubleRow,
                            )
                    else:
                        # last dkt: finish + drain blocks one at a time so the
                        # tails overlap the remaining matmuls
                        for i, (s, n) in enumerate(BLKS):
                            nc.tensor.matmul(
                                psums[i], sA, lbd[:, :, s : s + n],
                                start=False, stop=False, perf_mode=pm.DoubleRow,
                            )
                            nc.tensor.matmul(
                                psums[i], sB, hbd[:, :, s : s + n],
                                start=False, stop=True, perf_mode=pm.DoubleRow,
                            )
                            tail(i, s, n)

    nc.finalize()
    return nc


def _get_program():
    if "nc" not in _CACHE:
        _CACHE["nc"] = _build_program()
    return _CACHE["nc"]


def _host_prep(inp, quant_weight, scales, zeros):
    """Build per-core input maps (layout/precision prep, no dequant math)."""
    inp64 = np.asarray(inp, dtype=np.float64)
    a = np.ascontiguousarray(inp64[:, 0::2].T)  # [HALF, B] even-i (pairs l)
    b = np.ascontiguousarray(inp64[:, 1::2].T)  # [HALF, B] odd-i  (pairs h)
    a_hi, a_lo = _split_fp8(a)
    b_hi, b_lo = _split_fp8(b)

    def stat(hi, lo):
        # [HALF,B] -> [128, NDKT*2*64]: per dkt d, group g, cols
        # [hi(16) 0(16) lo(16) 0(16)] of j-rows d*256 + g*128 + p
        # (psum partition slices must be 32-aligned -> hi rows 0:16, lo 32:48)
        z = np.zeros((NDKT, 2, 128, B), dtype=FP8)
        m = np.concatenate(
            [hi.reshape(NDKT, 2, 128, B), z, lo.reshape(NDKT, 2, 128, B), z],
            axis=-1,
        )  # [NDKT, 2, 128, 64]
        return np.ascontiguousarray(
            m.transpose(2, 0, 1, 3).reshape(128, NDKT * 128)
        )

    statA = stat(a_hi, a_lo)
    statB = stat(b_hi, b_lo)
    # merged [128, NDKT*256]: per dkt, statA 128 cols then statB 128 cols
    stat_m = np.ascontiguousarray(
        np.concatenate(
            [statA.reshape(128, NDKT, 128), statB.reshape(128, NDKT, 128)],
            axis=-1,
        ).reshape(128, NDKT * 256)
    )

    rowsum = inp64.sum(axis=1)  # [B]
    rs_hi, rs_lo = _split_bf16(rowsum)
    corrL = np.zeros((4, 64), dtype=BF16)
    # stream values are nibble*2^-9 (subnormal path) or nibble+8 (offset
    # path); psum is scaled by P = 2^-9 or 1 accordingly
    s9 = np.float64(1.0 if USE_OFFSET else 2.0**-9)
    corrL[0, :B] = (rs_hi.astype(np.float64) * s9).astype(BF16)
    corrL[1, :B] = corrL[0, :B]
    corrL[2, :B] = (rs_lo.astype(np.float64) * s9).astype(BF16)
    corrL[3, :B] = corrL[2, :B]

    qw = np.asarray(quant_weight)
    scales = np.asarray(scales, dtype=np.float64).reshape(-1)
    zeros = np.asarray(zeros, dtype=np.float64).reshape(-1)

    in_maps = []
    for cidx in range(NCORES):
        rows = slice(cidx * OS, (cidx + 1) * OS)
        qc = np.ascontiguousarray(qw[rows].astype(np.uint8).T)  # [HALF, OS]
        # byte layout per partition: (gA[c0], gB[c0], gA[c1], gB[c1], ...)
        q_arr = np.ascontiguousarray(
            qc.reshape(NDKT, 2, 128, OS).transpose(0, 2, 3, 1).reshape(
                NDKT * 128, 2 * OS
            )
        ).view(np.uint32)
        z8 = zeros[rows] + (8.0 if USE_OFFSET else 0.0)
        z_hi, z_lo = _split_bf16(z8)
        corr_m = np.zeros((4, 64 + OS), dtype=BF16)
        corr_m[:, :64] = corrL
        corr_m[0, 64:] = -z_hi
        corr_m[1, 64:] = -z_lo
        corr_m[2, 64:] = -z_hi
        corr_m[3, 64:] = -z_lo
        sc_c = np.broadcast_to(
            (scales[rows] * (1.0 if USE_OFFSET else 512.0)).astype(np.float32),
            (B, OS),
        ).copy()
        in_maps.append({"q": q_arr, "stat": stat_m, "corr": corr_m, "sc": sc_c})
    return in_maps


def kernel(inp, quant_weight, scales, zeros):
    from concourse.bass_utils import run_bass_kernel_spmd

    nc = _get_program()
    in_maps = _host_prep(inp, quant_weight, scales, zeros)
    res = run_bass_kernel_spmd(nc, in_maps, core_ids=list(range(NCORES)))
    out = np.concatenate(
        [res.results[c]["out"] for c in range(NCORES)], axis=1
    )
    return np.ascontiguousarray(out.astype(np.float32))
